# revision 1
# baseline (speedup 1.0000x reference)
"""GATv2 message-passing kernel for 8 Trainium2 NeuronCores.

Strategy:
  - Targets sharded by node range: core k owns target nodes [12500k, 12500(k+1)).
  - Host (index-only preprocessing): per core, sort its incoming edges by
    target, order targets by in-degree, and lay edges out as degree-uniform
    tiles [128 target-nodes x d_t slots] so that segment max/sum become
    free-axis reductions and xr[tgt] becomes a per-partition broadcast.
  - Device: xl = x@Wl, xr = x@Wr computed from the core's node shard (PE),
    xl table AllGathered across cores; per-edge xl[src] fetched with
    per-column indirect DMAs; alpha/softmax/numerators on ACT+DVE.
  - Output rows assembled and inverse-permuted on host.
"""

import numpy as np

N_NODES = 100000
N_EDGES = 6400000
D_IN = 256
OUT = 2
NEG_SLOPE = 0.2
N_CORES = 8
NPC = N_NODES // N_CORES          # 12500 nodes per core
NT = 98                           # node tiles per core (98*128 = 12544)
NPAD = NT * 128                   # 12544
NTAB = NPAD * N_CORES             # 100352 rows in the gathered xl table
CHUNK_COLS = 768                  # target stream columns per processing chunk

_CACHE = {}


def _host_prep(x, edge_index, edge_attr, Wl, bl, Wr, br, We, att, bias):
    src = np.asarray(edge_index[0], dtype=np.int64)
    tgt = np.asarray(edge_index[1], dtype=np.int64)
    ea = np.asarray(edge_attr, dtype=np.float32).reshape(-1)
    x = np.asarray(x, dtype=np.float32)

    core_of = tgt // NPC
    tl = (tgt - core_of * NPC).astype(np.int64)

    deg_full = np.bincount(tgt, minlength=N_NODES)

    node_perm = np.zeros((N_CORES, NPAD), dtype=np.int64)
    rank_of = np.zeros((N_CORES, NPAD), dtype=np.int64)
    deg_sorted = np.zeros((N_CORES, NPAD), dtype=np.int64)
    for k in range(N_CORES):
        dk = np.zeros(NPAD, dtype=np.int64)
        dk[:NPC] = deg_full[k * NPC:(k + 1) * NPC]
        order = np.argsort(-dk, kind="stable")
        node_perm[k] = order
        rank_of[k][order] = np.arange(NPAD)
        deg_sorted[k] = dk[order]

    # common per-tile padded degree (same across cores so SPMD shapes match)
    tile_max = deg_sorted.reshape(N_CORES, NT, 128).max(axis=2).max(axis=0)
    d_t = np.maximum(((tile_max + 1) // 2) * 2, 2).astype(np.int64)
    col_off = np.concatenate([[0], np.cumsum(d_t)[:-1]])
    T = int(d_t.sum())

    # global xl-table row for each source node
    g_row = (src // NPC) * NPAD + (src % NPC)

    xT = np.ascontiguousarray(x.T)  # [256, N]

    in_maps = []
    W4 = np.concatenate([np.asarray(Wl, np.float32), np.asarray(Wr, np.float32)], axis=1)  # [256, 4]
    w4_sb = np.concatenate([W4[0:128, :], W4[128:256, :]], axis=1).astype(np.float32)  # [128, 8]

    for k in range(N_CORES):
        m = core_of == k
        e_src_row = g_row[m]
        e_r = rank_of[k][tl[m]]
        e_ea = ea[m]
        order = np.argsort(e_r, kind="stable")
        rs = e_r[order]
        cnt = deg_sorted[k]
        start = np.concatenate([[0], np.cumsum(cnt)[:-1]])
        slot = np.arange(len(rs)) - start[rs]
        t_idx = rs // 128
        p = rs % 128
        w = col_off[t_idx] + slot
        flat = (p * T + w).astype(np.int64)

        src_pp = np.zeros(128 * T, dtype=np.int32)
        ea_pp = np.zeros(128 * T, dtype=np.float32)
        mask_pp = np.full(128 * T, -1e30, dtype=np.float32)
        src_pp[flat] = e_src_row[order].astype(np.int32)
        ea_pp[flat] = e_ea[order]
        mask_pp[flat] = 0.0

        xTk = np.zeros((D_IN, NPAD), dtype=np.float32)
        xTk[:, :NPC] = xT[:, k * NPC:(k + 1) * NPC]

        tilerow = node_perm[k].reshape(NT, 128).T.astype(np.int32)  # [128, NT]

        in_maps.append({
            "xT": xTk,
            "w4": w4_sb,
            "src_pp": src_pp.reshape(128, T),
            "ea_pp": ea_pp.reshape(128, T),
            "mask_pp": mask_pp.reshape(128, T),
            "tilerow": np.ascontiguousarray(tilerow),
        })

    consts = dict(
        We0=float(np.asarray(We).reshape(-1)[0]), We1=float(np.asarray(We).reshape(-1)[1]),
        att0=float(np.asarray(att)[0]), att1=float(np.asarray(att)[1]),
        K0=float(np.asarray(bl)[0] + np.asarray(br)[0]),
        K1=float(np.asarray(bl)[1] + np.asarray(br)[1]),
        bl0=float(np.asarray(bl)[0]), bl1=float(np.asarray(bl)[1]),
        bias0=float(np.asarray(bias)[0]), bias1=float(np.asarray(bias)[1]),
    )
    meta = dict(T=T, d_t=tuple(int(v) for v in d_t), col_off=tuple(int(v) for v in col_off))
    return in_maps, consts, meta, node_perm


def _build(meta, consts, debug=False):
    from concourse import bacc, mybir
    import concourse.bass as bass
    import concourse.tile as tile

    T = meta["T"]
    d_t = meta["d_t"]
    col_off = meta["col_off"]
    f32 = mybir.dt.float32
    i32 = mybir.dt.int32
    AX = mybir.AxisListType.X
    ALU = mybir.AluOpType
    ACTF = mybir.ActivationFunctionType

    nc = bacc.Bacc("TRN2", target_bir_lowering=False, debug=False, num_devices=N_CORES)
    xT_d = nc.dram_tensor("xT", [D_IN, NPAD], f32, kind="ExternalInput").ap()
    w4_d = nc.dram_tensor("w4", [128, 8], f32, kind="ExternalInput").ap()
    src_d = nc.dram_tensor("src_pp", [128, T], i32, kind="ExternalInput").ap()
    ea_d = nc.dram_tensor("ea_pp", [128, T], f32, kind="ExternalInput").ap()
    mask_d = nc.dram_tensor("mask_pp", [128, T], f32, kind="ExternalInput").ap()
    tilerow_d = nc.dram_tensor("tilerow", [128, NT], i32, kind="ExternalInput").ap()
    out_d = nc.dram_tensor("outp", [NPAD, OUT], f32, kind="ExternalOutput").ap()
    if debug:
        dbg_xl = nc.dram_tensor("dbg_xl", [128, 2 * NT], f32, kind="ExternalOutput").ap()
        dbg_xr = nc.dram_tensor("dbg_xr", [128, 2 * NT], f32, kind="ExternalOutput").ap()
        dbg_xrg = nc.dram_tensor("dbg_xrg", [128, 2 * NT], f32, kind="ExternalOutput").ap()
        dbg_namax = nc.dram_tensor("dbg_namax", [128, NT], f32, kind="ExternalOutput").ap()
        dbg_denom = nc.dram_tensor("dbg_denom", [128, NT], f32, kind="ExternalOutput").ap()
        dbg_nume0 = nc.dram_tensor("dbg_nume0", [128, NT], f32, kind="ExternalOutput").ap()
        dbg_u = nc.dram_tensor("dbg_u", [128, 2 * meta["d_t"][0]], f32, kind="ExternalOutput").ap()
        dbg_alpha = nc.dram_tensor("dbg_alpha", [128, meta["d_t"][0]], f32, kind="ExternalOutput").ap()

    # chunk the tiles
    chunks = []
    t0 = 0
    while t0 < NT:
        t1 = t0
        w = 0
        while t1 < NT and (w + d_t[t1] <= CHUNK_COLS or t1 == t0):
            w += d_t[t1]
            t1 += 1
        chunks.append((t0, t1, col_off[t0], col_off[t0] + w))
        t0 = t1
    wmax = max(c[3] - c[2] for c in chunks)

    with tile.TileContext(nc) as tc:
        with tc.tile_pool(name="persist", bufs=1) as pp, \
             tc.tile_pool(name="stream", bufs=3) as sp, \
             tc.tile_pool(name="scratch", bufs=2) as scr, \
             tc.tile_pool(name="psum", bufs=2, space="PSUM") as psp, \
             tc.tile_pool(name="dram", bufs=1, space="DRAM") as dp:

            w4_sb = pp.tile([128, 8], f32)
            nc.sync.dma_start(out=w4_sb[:], in_=w4_d[:])
            tilerow_sb = pp.tile([128, NT], i32)
            nc.sync.dma_start(out=tilerow_sb[:], in_=tilerow_d[:])

            xl_slab_sb = pp.tile([128, 2 * NT], f32)
            xr_slab_sb = pp.tile([128, 2 * NT], f32)

            # ---- Phase A: xl/xr for this core's node shard ----
            for nb in range(NT):
                xa = sp.tile([128, 128], f32, tag="xa")
                xb = sp.tile([128, 128], f32, tag="xb")
                nc.sync.dma_start(out=xa[:], in_=xT_d[0:128, 128 * nb:128 * (nb + 1)])
                nc.sync.dma_start(out=xb[:], in_=xT_d[128:256, 128 * nb:128 * (nb + 1)])
                ps = psp.tile([128, 4], f32)
                nc.tensor.matmul(out=ps[:], lhsT=xa[:], rhs=w4_sb[:, 0:4], start=True, stop=False)
                nc.tensor.matmul(out=ps[:], lhsT=xb[:], rhs=w4_sb[:, 4:8], start=False, stop=True)
                nc.vector.tensor_copy(out=xl_slab_sb[:, 2 * nb:2 * nb + 2], in_=ps[:, 0:2])
                nc.vector.tensor_copy(out=xr_slab_sb[:, 2 * nb:2 * nb + 2], in_=ps[:, 2:4])

            xl_slab_d = dp.tile([NPAD, 2], f32)
            xr_slab_d = dp.tile([NPAD, 2], f32)
            # SBUF [p, (t c)] -> DRAM row 128*t+p
            nc.sync.dma_start(
                out=xl_slab_d[:].rearrange("(t p) c -> p t c", p=128),
                in_=xl_slab_sb[:].rearrange("p (t c) -> p t c", c=2),
            )
            nc.sync.dma_start(
                out=xr_slab_d[:].rearrange("(t p) c -> p t c", p=128),
                in_=xr_slab_sb[:].rearrange("p (t c) -> p t c", c=2),
            )

            xl_full = dp.tile([NTAB, 2], f32)
            nc.gpsimd.collective_compute(
                "AllGather",
                mybir.AluOpType.bypass,
                replica_groups=[list(range(N_CORES))],
                ins=[xl_slab_d.opt()],
                outs=[xl_full.opt()],
            )
            # Explicitly consume the collective output and fence: the
            # indirect-DMA readers of xl_full must not race the AllGather.
            tok = pp.tile([128, 2], f32)
            nc.sync.dma_start(out=tok[:], in_=xl_full[NTAB - 128:NTAB, :])
            tc.strict_bb_all_engine_barrier()

            # ---- xr per tile (local gather from xr_slab_d) ----
            xr_sb = pp.tile([128, 2 * NT], f32)
            for t in range(NT):
                nc.gpsimd.indirect_dma_start(
                    out=xr_sb[:, 2 * t:2 * t + 2],
                    out_offset=None,
                    in_=xr_slab_d[:],
                    in_offset=bass.IndirectOffsetOnAxis(ap=tilerow_sb[:, t:t + 1], axis=0),
                )

            # ---- Phase C: edge chunks ----
            namax = pp.tile([128, NT], f32)
            denom = pp.tile([128, NT], f32)
            nume0 = pp.tile([128, NT], f32)
            nume1 = pp.tile([128, NT], f32)

            for (ct0, ct1, c0, c1) in chunks:
                Wc = c1 - c0
                src_c = sp.tile([128, wmax], i32, tag="src")
                ea_c = sp.tile([128, wmax], f32, tag="ea")
                mask_c = sp.tile([128, wmax], f32, tag="mask")
                nc.sync.dma_start(out=src_c[:, :Wc], in_=src_d[:, c0:c1])
                nc.sync.dma_start(out=ea_c[:, :Wc], in_=ea_d[:, c0:c1])
                nc.sync.dma_start(out=mask_c[:, :Wc], in_=mask_d[:, c0:c1])

                u_c = sp.tile([128, 2 * wmax], f32, tag="u")
                for w in range(Wc):
                    nc.gpsimd.indirect_dma_start(
                        out=u_c[:, 2 * w:2 * w + 2],
                        out_offset=None,
                        in_=xl_full[:],
                        in_offset=bass.IndirectOffsetOnAxis(ap=src_c[:, w:w + 1], axis=0),
                    )
                u0 = u_c[:, 0:2 * Wc].rearrange("p (w c) -> p w c", c=2)[:, :, 0:1].rearrange("p w one -> p (w one)")
                u1 = u_c[:, 0:2 * Wc].rearrange("p (w c) -> p w c", c=2)[:, :, 1:2].rearrange("p w one -> p (w one)")

                v0 = scr.tile([128, wmax], f32, tag="v0")
                v1 = scr.tile([128, wmax], f32, tag="v1")
                # v_c = ea*We_c + K_c
                nc.vector.tensor_scalar(out=v0[:, :Wc], in0=ea_c[:, :Wc],
                                        scalar1=consts["We0"], scalar2=consts["K0"],
                                        op0=ALU.mult, op1=ALU.add)
                nc.vector.tensor_scalar(out=v1[:, :Wc], in0=ea_c[:, :Wc],
                                        scalar1=consts["We1"], scalar2=consts["K1"],
                                        op0=ALU.mult, op1=ALU.add)
                nc.vector.tensor_tensor(out=v0[:, :Wc], in0=v0[:, :Wc], in1=u0, op=ALU.add)
                nc.vector.tensor_tensor(out=v1[:, :Wc], in0=v1[:, :Wc], in1=u1, op=ALU.add)

                lr0 = scr.tile([128, wmax], f32, tag="lr0")
                lr1 = scr.tile([128, wmax], f32, tag="lr1")
                # m_c = v_c + xr_c[tile node] (per-partition broadcast), in-place into v
                for t in range(ct0, ct1):
                    lo = col_off[t] - c0
                    hi = lo + d_t[t]
                    w_t = hi - lo
                    nc.vector.tensor_tensor(out=v0[:, lo:hi], in0=v0[:, lo:hi],
                                            in1=xr_sb[:, 2 * t:2 * t + 1].to_broadcast([128, w_t]),
                                            op=ALU.add)
                    nc.vector.tensor_tensor(out=v1[:, lo:hi], in0=v1[:, lo:hi],
                                            in1=xr_sb[:, 2 * t + 1:2 * t + 2].to_broadcast([128, w_t]),
                                            op=ALU.add)
                # leaky relu: lr = max(m, 0.2*m)  (ACT Lrelu ignores its alpha param on HW)
                nc.vector.tensor_scalar_mul(lr0[:, :Wc], v0[:, :Wc], NEG_SLOPE)
                nc.vector.tensor_tensor(out=lr0[:, :Wc], in0=lr0[:, :Wc], in1=v0[:, :Wc], op=ALU.max)
                nc.vector.tensor_scalar_mul(lr1[:, :Wc], v1[:, :Wc], NEG_SLOPE)
                nc.vector.tensor_tensor(out=lr1[:, :Wc], in0=lr1[:, :Wc], in1=v1[:, :Wc], op=ALU.max)

                alpha = scr.tile([128, wmax], f32, tag="alpha")
                nc.vector.tensor_scalar(out=lr0[:, :Wc], in0=lr0[:, :Wc], scalar1=consts["att0"],
                                        scalar2=None, op0=ALU.mult)
                nc.vector.tensor_scalar(out=lr1[:, :Wc], in0=lr1[:, :Wc], scalar1=consts["att1"],
                                        scalar2=None, op0=ALU.mult)
                nc.vector.tensor_tensor(out=alpha[:, :Wc], in0=lr0[:, :Wc], in1=lr1[:, :Wc], op=ALU.add)
                nc.vector.tensor_tensor(out=alpha[:, :Wc], in0=alpha[:, :Wc], in1=mask_c[:, :Wc], op=ALU.add)

                if debug and ct0 == 0:
                    nc.sync.dma_start(out=dbg_u[:], in_=u_c[:, 0:2 * d_t[0]])
                    nc.sync.dma_start(out=dbg_alpha[:], in_=alpha[:, 0:d_t[0]])

                ex = scr.tile([128, wmax], f32, tag="ex")
                for t in range(ct0, ct1):
                    lo = col_off[t] - c0
                    hi = lo + d_t[t]
                    nc.vector.tensor_reduce(out=namax[:, t:t + 1], in_=alpha[:, lo:hi],
                                            axis=AX, op=ALU.max, negate=True)
                    nc.vector.tensor_scalar_min(namax[:, t:t + 1], namax[:, t:t + 1], 30.0)
                    nc.scalar.activation(out=ex[:, lo:hi], in_=alpha[:, lo:hi], func=ACTF.Exp,
                                         bias=namax[:, t:t + 1], scale=1.0,
                                         accum_out=denom[:, t:t + 1])

                # ex * u
                nc.vector.tensor_tensor(out=v0[:, :Wc], in0=ex[:, :Wc], in1=u0, op=ALU.mult)
                nc.vector.tensor_tensor(out=v1[:, :Wc], in0=ex[:, :Wc], in1=u1, op=ALU.mult)
                for t in range(ct0, ct1):
                    lo = col_off[t] - c0
                    hi = lo + d_t[t]
                    nc.vector.tensor_reduce(out=nume0[:, t:t + 1], in_=v0[:, lo:hi], axis=AX, op=ALU.add)
                    nc.vector.tensor_reduce(out=nume1[:, t:t + 1], in_=v1[:, lo:hi], axis=AX, op=ALU.add)

            if debug:
                dbg_xlf = nc.dram_tensor("dbg_xlf", [256, 2], f32, kind="ExternalOutput").ap()
                xlf_sb = pp.tile([128, 4], f32)
                nc.sync.dma_start(out=xlf_sb[:, 0:2], in_=xl_full[0:128, :])
                nc.sync.dma_start(out=xlf_sb[:, 2:4], in_=xl_full[NPAD:NPAD + 128, :])
                nc.sync.dma_start(out=dbg_xlf[0:128, :], in_=xlf_sb[:, 0:2])
                nc.sync.dma_start(out=dbg_xlf[128:256, :], in_=xlf_sb[:, 2:4])
                nc.sync.dma_start(out=dbg_xl[:], in_=xl_slab_sb[:])
                nc.sync.dma_start(out=dbg_xr[:], in_=xr_slab_sb[:])
                nc.sync.dma_start(out=dbg_xrg[:], in_=xr_sb[:])
                nc.sync.dma_start(out=dbg_namax[:], in_=namax[:])
                nc.sync.dma_start(out=dbg_denom[:], in_=denom[:])
                nc.sync.dma_start(out=dbg_nume0[:], in_=nume0[:])

            # ---- Phase D: finish ----
            outsb = pp.tile([128, 2 * NT], f32)
            dn = pp.tile([128, NT], f32)
            nc.vector.tensor_scalar_add(dn[:], denom[:], 1e-16)
            o0 = outsb[:].rearrange("p (t c) -> p t c", c=2)[:, :, 0:1].rearrange("p t one -> p (t one)")
            o1 = outsb[:].rearrange("p (t c) -> p t c", c=2)[:, :, 1:2].rearrange("p t one -> p (t one)")
            if consts["bl0"] != 0.0 or consts["bl1"] != 0.0:
                tmpb = pp.tile([128, NT], f32)
                nc.vector.tensor_scalar_mul(tmpb[:], denom[:], consts["bl0"])
                nc.vector.tensor_tensor(out=nume0[:], in0=nume0[:], in1=tmpb[:], op=ALU.add)
                nc.vector.tensor_scalar_mul(tmpb[:], denom[:], consts["bl1"])
                nc.vector.tensor_tensor(out=nume1[:], in0=nume1[:], in1=tmpb[:], op=ALU.add)
            nc.vector.reciprocal(out=dn[:], in_=dn[:])
            nc.vector.tensor_tensor(out=o0, in0=nume0[:], in1=dn[:], op=ALU.mult)
            nc.vector.tensor_tensor(out=o1, in0=nume1[:], in1=dn[:], op=ALU.mult)
            if consts["bias0"] != 0.0:
                nc.vector.tensor_scalar_add(o0, o0, consts["bias0"])
            if consts["bias1"] != 0.0:
                nc.vector.tensor_scalar_add(o1, o1, consts["bias1"])

            nc.sync.dma_start(
                out=out_d[:].rearrange("(t p) c -> p t c", p=128),
                in_=outsb[:].rearrange("p (t c) -> p t c", c=2),
            )

    nc.compile()
    return nc


def kernel(**inputs) -> np.ndarray:
    from concourse.bass_utils import run_bass_kernel_spmd

    in_maps, consts, meta, node_perm = _host_prep(**inputs)
    key = (meta["T"], meta["d_t"],
           tuple(sorted(consts.items())))
    if key not in _CACHE:
        _CACHE.clear()
        _CACHE[key] = _build(meta, consts)
    nc = _CACHE[key]

    res = run_bass_kernel_spmd(nc, in_maps, list(range(N_CORES)))

    out = np.zeros((N_NODES, OUT), dtype=np.float32)
    for k in range(N_CORES):
        slab = res.results[k]["outp"]  # [NPAD, 2], row r = rank r
        perm = node_perm[k]            # rank -> local node id
        valid = perm < NPC
        out[k * NPC + perm[valid]] = slab[valid]
    return out

